# revision 1
# baseline (speedup 1.0000x reference)
"""Causal self-attention (B=2, T=2048, C=1024, H=16) on 8 trn2 NeuronCores.

Sharding: data-parallel over batch (2 groups of 4 cores) x tensor-parallel over
heads (4 heads / core). Each core computes Q^T/K^T in [ch, tok] layout and V in
[tok, ch] layout from a host-pre-transposed x slice, runs causal attention for
its 4 heads with the softmax denominator folded into the PV matmul via a ones
column, applies its slice of the output projection, and the partial projections
are summed with an on-device ReduceScatter over each 4-core group.
"""

import sys

for _p in ("/opt/trn_rl_repo",):
    if _p not in sys.path:
        sys.path.append(_p)

import numpy as np
from contextlib import ExitStack

import concourse.bass as bass
import concourse.mybir as mybir
import concourse.tile as tile
from concourse import bass_utils

B, T, C, H = 2, 2048, 1024, 16
D = C // H              # 64
N_CORES = 8
GROUPS = [[0, 1, 2, 3], [4, 5, 6, 7]]
HL = 4                  # heads per core
CL = HL * D             # 256 local channels
KC = C // 128           # 8 contraction chunks of 128
NT = T // 512           # 4 token chunks of 512
TOKC = T // 128         # 16 token chunks of 128
F32 = mybir.dt.float32
F32R = mybir.dt.float32r


def _legalize_waits(nc):
    """This walrus build allows at most ONE sync-wait per instruction. Move
    extra waits onto same-engine NoOps inserted just before the instruction."""
    n_split = 0
    for f in nc.m.functions:
        for bb in f.blocks:
            out = []
            for inst in bb.instructions:
                si = inst.sync_info
                waits = list(si.on_wait) if si is not None and si.on_wait else []
                if len(waits) > 1:
                    for i, w in enumerate(waits[:-1]):
                        out.append(
                            mybir.InstNoOp(
                                name=f"wsplit_{inst.name}_{i}",
                                engine=inst.engine,
                                ins=[],
                                outs=[],
                                sync_info=mybir.SyncInfo(on_wait=[w], on_update=[]),
                            )
                        )
                        n_split += 1
                    si.on_wait = [waits[-1]]
                out.append(inst)
            bb.instructions = out
    return n_split


def _build_bass():
    nc = bass.Bass("TRN2", target_bir_lowering=False, debug=False,
                   num_devices=N_CORES)

    xT = nc.dram_tensor("xT", [C, T], F32R, kind="ExternalInput").ap()
    w_qk = nc.dram_tensor("w_qk", [C, 2 * CL], F32R, kind="ExternalInput").ap()
    b_qk = nc.dram_tensor("b_qk", [2 * CL], F32, kind="ExternalInput").ap()
    w_v = nc.dram_tensor("w_v", [C, CL], F32R, kind="ExternalInput").ap()
    b_v = nc.dram_tensor("b_v", [CL], F32R, kind="ExternalInput").ap()
    w_pr = nc.dram_tensor("w_pr", [CL, C], F32R, kind="ExternalInput").ap()
    b_pr = nc.dram_tensor("b_pr", [C], F32, kind="ExternalInput").ap()
    out_rs = nc.dram_tensor("out_rs", [C // 4, T], F32, kind="ExternalOutput").ap()

    with tile.TileContext(nc) as tc:
        with ExitStack() as ctx:
            with nc.allow_low_precision(reason="float32r is 4-byte; full fp32 accumulate"):
                _build_body(ctx, tc, nc, xT, w_qk, b_qk, w_v, b_v, w_pr, b_pr, out_rs)

    _legalize_waits(nc)
    return nc


def _build_body(ctx, tc, nc, xT, w_qk, b_qk, w_v, b_v, w_pr, b_pr, out_rs):
    Identity = mybir.ActivationFunctionType.Identity
    Exp = mybir.ActivationFunctionType.Exp

    persist = ctx.enter_context(tc.tile_pool(name="persist", bufs=1))
    dram = ctx.enter_context(tc.tile_pool(name="dram", bufs=1, space="DRAM"))

    # ---- constant / bias tiles -------------------------------------------
    b_qk_sb = persist.tile([128, 4], F32, name="b_qk_sb")
    nc.sync.dma_start(b_qk_sb[:], b_qk.rearrange("(m p) -> p m", p=128))
    b_pr_sb = persist.tile([128, 8], F32, name="b_pr_sb")
    nc.sync.dma_start(b_pr_sb[:], b_pr.rearrange("(m p) -> p m", p=128))
    b_v_row = persist.tile([1, CL], F32R, name="b_v_row")
    nc.sync.dma_start(b_v_row[:], b_v.rearrange("(a c) -> a c", a=1))
    ones_sb = persist.tile([128, HL], F32, name="ones_sb")
    nc.gpsimd.memset(ones_sb[:], 1.0)
    ones_f32 = persist.tile([1, 128], F32, name="ones_f32")
    nc.gpsimd.memset(ones_f32[:], 1.0)
    ones_row = persist.tile([1, 128], F32R, name="ones_row")
    nc.vector.tensor_copy(ones_row[:], ones_f32[:])
    # broadcast b_v across partitions via a rank-1 matmul
    b_v_bc = persist.tile([128, CL], F32, name="b_v_bc")
    with tc.tile_pool(name="psInit", bufs=1, space="PSUM") as psI:
        bvp = psI.tile([128, CL], F32)
        nc.tensor.matmul(bvp[:], lhsT=ones_row[:], rhs=b_v_row[:],
                         start=True, stop=True)
        nc.vector.tensor_copy(b_v_bc[:], bvp[:])
    # 0/1 causal masks for the 4 diagonal-band offsets: mask_r[x, y] = y-x-128r >= 0
    mask_sb = []
    for r in range(4):
        m = persist.tile([128, 512], F32, name=f"mask_{r}")
        nc.gpsimd.memset(m[:], 1.0)
        nc.gpsimd.affine_select(
            out=m[:], in_=m[:], compare_op=mybir.AluOpType.is_ge, fill=0.0,
            base=-128 * r, pattern=[[1, 512]], channel_multiplier=-1,
        )
        mask_sb.append(m)

    # ---- weight + activation loads ---------------------------------------
    w_qk_sb = []
    for kc in range(KC):
        t = persist.tile([128, 2 * CL], F32R, name=f"w_qk_{kc}")
        nc.sync.dma_start(t[:], w_qk[kc * 128:(kc + 1) * 128, :])
        w_qk_sb.append(t)
    w_v_sb = []
    for kc in range(KC):
        t = persist.tile([128, CL], F32R, name=f"w_v_{kc}")
        nc.sync.dma_start(t[:], w_v[kc * 128:(kc + 1) * 128, :])
        w_v_sb.append(t)
    w_pr_sb = []
    for kc in range(2):
        t = persist.tile([128, C], F32R, name=f"w_pr_{kc}")
        nc.sync.dma_start(t[:], w_pr[kc * 128:(kc + 1) * 128, :])
        w_pr_sb.append(t)
    xT_sb = {}
    for n in range(NT):
        for kc in range(KC):
            t = persist.tile([128, 512], F32R, name=f"xT_{kc}_{n}")
            nc.sync.dma_start(
                t[:], xT[kc * 128:(kc + 1) * 128, n * 512:(n + 1) * 512])
            xT_sb[kc, n] = t

    # ---- persistent intermediates ----------------------------------------
    # QK_sb[m]: m=0,1 -> Q channels (heads 0,1 | 2,3), m=2,3 -> K channels
    QK_sb = [persist.tile([128, T], F32R, name=f"QK_{m}") for m in range(4)]
    # V in [tok, ch] layout, 65 cols/head: col h*65 is the ones column
    V_sb = [persist.tile([128, HL * 65], F32R, name=f"V_{t}") for t in range(TOKC)]
    yT_sb = [persist.tile([128, T], F32R, name=f"yT_{i}") for i in range(2)]

    # ---- phase A: Q^T / K^T gemm  (out[ch, tok]) -------------------------
    with ExitStack() as pctx:
        psA = pctx.enter_context(tc.tile_pool(name="psA", bufs=4, space="PSUM"))
        psB = pctx.enter_context(tc.tile_pool(name="psB", bufs=4, space="PSUM"))
        for m in range(4):
            for n in range(NT):
                ps = psA.tile([128, 512], F32)
                for kc in range(KC):
                    nc.tensor.matmul(
                        ps[:], lhsT=w_qk_sb[kc][:, m * 128:(m + 1) * 128],
                        rhs=xT_sb[kc, n][:], start=(kc == 0), stop=(kc == KC - 1))
                nc.scalar.activation(
                    QK_sb[m][:, n * 512:(n + 1) * 512], ps[:], Identity,
                    bias=b_qk_sb[:, m:m + 1])

        # ---- phase B: V gemm (out[tok, ch]) + bias + ones column ---------
        for n in range(NT):
            for t in range(4):
                ps = psB.tile([128, CL], F32)
                for kc in range(KC):
                    nc.tensor.matmul(
                        ps[:], lhsT=xT_sb[kc, n][:, t * 128:(t + 1) * 128],
                        rhs=w_v_sb[kc][:], start=(kc == 0), stop=(kc == KC - 1))
                tok = n * 4 + t
                vt = V_sb[tok][:].rearrange("p (h c) -> p h c", h=HL)
                nc.vector.tensor_add(
                    vt[:, :, 0:64], ps[:].rearrange("p (h c) -> p h c", h=HL),
                    b_v_bc[:].rearrange("p (h c) -> p h c", h=HL))
                nc.vector.tensor_copy(vt[:, :, 64:65].rearrange("p h c -> p (h c)"),
                                      ones_sb[:])

    # ---- phase C: attention ----------------------------------------------
    def q_ap(h):
        return QK_sb[h // 2][(h % 2) * 64:(h % 2) * 64 + 64, :]

    def k_ap(h):
        return QK_sb[2 + h // 2][(h % 2) * 64:(h % 2) * 64 + 64, :]

    with ExitStack() as pctx:
        psS = pctx.enter_context(tc.tile_pool(name="psS", bufs=5, space="PSUM"))
        psY = pctx.enter_context(tc.tile_pool(name="psY", bufs=2, space="PSUM"))
        psBC = pctx.enter_context(tc.tile_pool(name="psBC", bufs=1, space="PSUM"))
        pP = pctx.enter_context(tc.tile_pool(name="pP", bufs=8))
        pT = pctx.enter_context(tc.tile_pool(name="pT", bufs=4))
        pR = pctx.enter_context(tc.tile_pool(name="pR", bufs=2))
        pBC = pctx.enter_context(tc.tile_pool(name="pBC", bufs=2))
        for qi in range(NT):
            for h in range(HL):
                nch = 4 * (qi + 1)
                y_ps = psY.tile([65, 512], F32)
                for j in range(nch):
                    s_ps = psS.tile([128, 512], F32)
                    nc.tensor.matmul(
                        s_ps[:], lhsT=k_ap(h)[:, j * 128:(j + 1) * 128],
                        rhs=q_ap(h)[:, qi * 512:(qi + 1) * 512],
                        start=True, stop=True)
                    rel = j - 4 * qi
                    if rel < 0:
                        p_sb = pP.tile([128, 512], F32R)
                        nc.scalar.activation(p_sb[:], s_ps[:], Exp, scale=0.125)
                    else:
                        p_tmp = pT.tile([128, 512], F32)
                        nc.scalar.activation(p_tmp[:], s_ps[:], Exp, scale=0.125)
                        p_sb = pP.tile([128, 512], F32R)
                        nc.vector.tensor_mul(p_sb[:], p_tmp[:], mask_sb[rel][:])
                    nc.tensor.matmul(
                        y_ps[:], lhsT=V_sb[j][:, h * 65:(h + 1) * 65],
                        rhs=p_sb[:], start=(j == 0), stop=(j == nch - 1))
                rec = pR.tile([1, 512], F32R)
                nc.vector.reciprocal(rec[:], y_ps[64:65, :])
                bc = psBC.tile([64, 512], F32)
                nc.tensor.matmul(bc[:], lhsT=ones_row[:, 0:64], rhs=rec[:],
                                 start=True, stop=True)
                bc_sb = pBC.tile([64, 512], F32)
                nc.scalar.copy(bc_sb[:], bc[:])
                nc.vector.tensor_mul(
                    yT_sb[h // 2][(h % 2) * 64:(h % 2) * 64 + 64,
                                  qi * 512:(qi + 1) * 512],
                    y_ps[0:64, :], bc_sb[:])

    # ---- phase D: output projection --------------------------------------
    bounce_in = dram.tile([C, T], F32, name="bounce_in")
    bounce_rs = dram.tile([C // 4, T], F32, name="bounce_rs")
    with ExitStack() as pctx:
        psD = pctx.enter_context(tc.tile_pool(name="psD", bufs=4, space="PSUM"))
        pO = pctx.enter_context(tc.tile_pool(name="pO", bufs=4))
        for n in range(NT):
            for m in range(8):
                ps = psD.tile([128, 512], F32)
                for kc in range(2):
                    nc.tensor.matmul(
                        ps[:], lhsT=w_pr_sb[kc][:, m * 128:(m + 1) * 128],
                        rhs=yT_sb[kc][:, n * 512:(n + 1) * 512],
                        start=(kc == 0), stop=(kc == 1))
                o_sb = pO.tile([128, 512], F32)
                nc.scalar.activation(o_sb[:], ps[:], Identity,
                                     bias=b_pr_sb[:, m:m + 1])
                nc.sync.dma_start(
                    bounce_in[m * 128:(m + 1) * 128, n * 512:(n + 1) * 512],
                    o_sb[:])

    # ---- phase E: reduce over the 4-core group, emit this rank's slice ---
    nc.gpsimd.collective_compute(
        "ReduceScatter", mybir.AluOpType.add, replica_groups=GROUPS,
        ins=[bounce_in[:]], outs=[bounce_rs[:]])
    nc.sync.dma_start(out_rs[:], bounce_rs[:])


_NC_CACHE = None


def _get_nc():
    global _NC_CACHE
    if _NC_CACHE is None:
        _NC_CACHE = _build_bass()
    return _NC_CACHE


def kernel(x, w_qkv, b_qkv, w_proj, b_proj, **_kw):
    x = np.asarray(x, dtype=np.float32)
    w_qkv = np.asarray(w_qkv, dtype=np.float32)
    b_qkv = np.asarray(b_qkv, dtype=np.float32)
    w_proj = np.asarray(w_proj, dtype=np.float32)
    b_proj = np.asarray(b_proj, dtype=np.float32)

    nc = _get_nc()
    in_maps = []
    for c in range(N_CORES):
        b = c // 4
        g = c % 4
        qs = slice(g * CL, (g + 1) * CL)
        ks = slice(C + g * CL, C + (g + 1) * CL)
        vs = slice(2 * C + g * CL, 2 * C + (g + 1) * CL)
        in_maps.append({
            "xT": np.ascontiguousarray(x[b].T),
            "w_qk": np.ascontiguousarray(
                np.concatenate([w_qkv[:, qs], w_qkv[:, ks]], axis=1)),
            "b_qk": np.ascontiguousarray(
                np.concatenate([b_qkv[qs], b_qkv[ks]])),
            "w_v": np.ascontiguousarray(w_qkv[:, vs]),
            "b_v": np.ascontiguousarray(b_qkv[vs]),
            "w_pr": np.ascontiguousarray(w_proj[g * CL:(g + 1) * CL, :]),
            "b_pr": b_proj if g == 0 else np.zeros_like(b_proj),
        })

    global _last_in_maps
    _last_in_maps = in_maps
    res = bass_utils.run_bass_kernel_spmd(nc, in_maps, core_ids=list(range(N_CORES)))

    out = np.empty((B, T, C), dtype=np.float32)
    for b in range(B):
        projT = np.concatenate(
            [res.results[4 * b + r]["out_rs"] for r in range(4)], axis=0)
        out[b] = projT.T
    return out


if __name__ == "__main__":
    rng = np.random.RandomState(0)
    ins = {
        "x": rng.randn(B, T, C).astype(np.float32),
        "w_qkv": rng.randn(C, 3 * C).astype(np.float32) / 32,
        "b_qkv": rng.randn(3 * C).astype(np.float32) / 32,
        "w_proj": rng.randn(C, C).astype(np.float32) / 32,
        "b_proj": rng.randn(C).astype(np.float32) / 32,
    }
    y = kernel(**ins)
    print("kernel ran, out shape", y.shape)



# revision 4
# speedup vs baseline: 1.2360x; 1.2360x over previous
"""Causal self-attention (B=2, T=2048, C=1024, H=16) on 8 trn2 NeuronCores.

Sharding: data-parallel over batch (2 groups of 4 cores) x tensor-parallel over
heads (4 heads / core). Per-token-chunk pipeline: for each 512-token chunk the
core computes Q^T/K^T ([ch,tok]) and V ([tok,ch]) slices, causal attention for
its 4 heads (softmax denominator folded into the PV matmul via a ones column),
its slice of the output projection, and a chunked ReduceScatter over the 4-core
group that overlaps the next chunk's compute. Diagonal attention blocks are
computed over a restricted q-window with the triangle masked by a small DVE
multiply, so no full-width mask multiplies and no wasted exp columns.
"""

import sys

for _p in ("/opt/trn_rl_repo",):
    if _p not in sys.path:
        sys.path.append(_p)

import numpy as np
from contextlib import ExitStack

import concourse.bass as bass
import concourse.mybir as mybir
import concourse.tile as tile
from concourse import bass_utils

B, T, C, H = 2, 2048, 1024, 16
D = C // H              # 64
N_CORES = 8
GROUPS = [[0, 1, 2, 3], [4, 5, 6, 7]]
HL = 4                  # heads per core
CL = HL * D             # 256 local channels
KC = C // 128           # 8 contraction chunks of 128
NT = T // 512           # 4 token chunks of 512
F32 = mybir.dt.float32
F32R = mybir.dt.float32r


def _legalize_waits(nc):
    """This walrus build allows at most ONE sync-wait per instruction. Move
    extra waits onto same-engine NoOps inserted just before the instruction."""
    n_split = 0
    for f in nc.m.functions:
        for bb in f.blocks:
            out = []
            for inst in bb.instructions:
                si = inst.sync_info
                waits = list(si.on_wait) if si is not None and si.on_wait else []
                if len(waits) > 1:
                    for i, w in enumerate(waits[:-1]):
                        out.append(
                            mybir.InstNoOp(
                                name=f"wsplit_{inst.name}_{i}",
                                engine=inst.engine,
                                ins=[],
                                outs=[],
                                sync_info=mybir.SyncInfo(on_wait=[w], on_update=[]),
                            )
                        )
                        n_split += 1
                    si.on_wait = [waits[-1]]
                out.append(inst)
            bb.instructions = out
    return n_split


def _build_bass():
    nc = bass.Bass("TRN2", target_bir_lowering=False, debug=False,
                   num_devices=N_CORES)

    xT = nc.dram_tensor("xT", [C, T], F32R, kind="ExternalInput").ap()
    w_qk = nc.dram_tensor("w_qk", [C, 2 * CL], F32R, kind="ExternalInput").ap()
    b_qk = nc.dram_tensor("b_qk", [2 * CL], F32, kind="ExternalInput").ap()
    w_v = nc.dram_tensor("w_v", [C, CL], F32R, kind="ExternalInput").ap()
    b_v = nc.dram_tensor("b_v", [CL], F32R, kind="ExternalInput").ap()
    w_pr = nc.dram_tensor("w_pr", [CL, C], F32R, kind="ExternalInput").ap()
    b_pr = nc.dram_tensor("b_pr", [C], F32, kind="ExternalInput").ap()
    out_rs = nc.dram_tensor("out_rs", [C // 4, T], F32, kind="ExternalOutput").ap()

    with tile.TileContext(nc) as tc:
        with ExitStack() as ctx:
            with nc.allow_low_precision(reason="float32r is 4-byte; full fp32 accumulate"):
                _build_body(ctx, tc, nc, xT, w_qk, b_qk, w_v, b_v, w_pr, b_pr, out_rs)

    _legalize_waits(nc)
    return nc


def _build_body(ctx, tc, nc, xT, w_qk, b_qk, w_v, b_v, w_pr, b_pr, out_rs):
    Exp = mybir.ActivationFunctionType.Exp

    persist = ctx.enter_context(tc.tile_pool(name="persist", bufs=1))
    dram = ctx.enter_context(tc.tile_pool(name="dram", bufs=1, space="DRAM"))

    # ---- constant / bias tiles -------------------------------------------
    b_qk_sb = persist.tile([128, 4], F32, name="b_qk_sb")
    nc.sync.dma_start(b_qk_sb[:], b_qk.rearrange("(m p) -> p m", p=128))
    b_pr_sb = persist.tile([128, 8], F32, name="b_pr_sb")
    nc.sync.dma_start(b_pr_sb[:], b_pr.rearrange("(m p) -> p m", p=128))
    b_v_row = persist.tile([1, CL], F32R, name="b_v_row")
    nc.sync.dma_start(b_v_row[:], b_v.rearrange("(a c) -> a c", a=1))
    ones_sb = persist.tile([128, HL], F32, name="ones_sb")
    nc.gpsimd.memset(ones_sb[:], 1.0)
    ones_f32 = persist.tile([1, 128], F32, name="ones_f32")
    nc.gpsimd.memset(ones_f32[:], 1.0)
    ones_row = persist.tile([1, 128], F32R, name="ones_row")
    nc.vector.tensor_copy(ones_row[:], ones_f32[:])
    # broadcast b_v across partitions via a rank-1 matmul
    b_v_bc = persist.tile([128, CL], F32, name="b_v_bc")
    with tc.tile_pool(name="psInit", bufs=1, space="PSUM") as psI:
        bvp = psI.tile([128, CL], F32)
        nc.tensor.matmul(bvp[:], lhsT=ones_row[:], rhs=b_v_row[:],
                         start=True, stop=True)
        nc.vector.tensor_copy(b_v_bc[:], bvp[:])
    # 0/1 masks for diagonal blocks:
    #   M_tri[x, y] = y - x >= 0            (128x128 lower-left triangle)
    #   M_zt[x, y]  = y - x - 128 >= 0      (zeros strip | triangle, 128x256)
    M_tri = persist.tile([128, 128], F32, name="M_tri")
    nc.gpsimd.memset(M_tri[:], 1.0)
    nc.gpsimd.affine_select(
        out=M_tri[:], in_=M_tri[:], compare_op=mybir.AluOpType.is_ge, fill=0.0,
        base=0, pattern=[[1, 128]], channel_multiplier=-1)
    M_zt = persist.tile([128, 256], F32, name="M_zt")
    nc.gpsimd.memset(M_zt[:], 1.0)
    nc.gpsimd.affine_select(
        out=M_zt[:], in_=M_zt[:], compare_op=mybir.AluOpType.is_ge, fill=0.0,
        base=-128, pattern=[[1, 256]], channel_multiplier=-1)

    # ---- weight + activation loads (xT chunk 0 first so PE starts early) --
    xT_sb = {}

    def load_x_chunk(n):
        for kc in range(KC):
            t = persist.tile([128, 512], F32R, name=f"xT_{kc}_{n}")
            nc.sync.dma_start(
                t[:], xT[kc * 128:(kc + 1) * 128, n * 512:(n + 1) * 512])
            xT_sb[kc, n] = t

    load_x_chunk(0)
    w_qk_sb = []
    for kc in range(KC):
        t = persist.tile([128, 2 * CL], F32R, name=f"w_qk_{kc}")
        nc.sync.dma_start(t[:], w_qk[kc * 128:(kc + 1) * 128, :])
        w_qk_sb.append(t)
    w_v_sb = []
    for kc in range(KC):
        t = persist.tile([128, CL], F32R, name=f"w_v_{kc}")
        nc.sync.dma_start(t[:], w_v[kc * 128:(kc + 1) * 128, :])
        w_v_sb.append(t)
    w_pr_sb = []
    for kc in range(2):
        t = persist.tile([128, C], F32R, name=f"w_pr_{kc}")
        nc.sync.dma_start(t[:], w_pr[kc * 128:(kc + 1) * 128, :])
        w_pr_sb.append(t)
    for n in range(1, NT):
        load_x_chunk(n)

    # ---- persistent intermediates ----------------------------------------
    # QK_sb[m]: m=0,1 -> Q channels (heads 0,1 | 2,3), m=2,3 -> K channels
    QK_sb = [persist.tile([128, T], F32R, name=f"QK_{m}") for m in range(4)]
    # V in [tok, ch] layout, 65 cols/head: col h*65+64 is the ones column
    V_sb = [persist.tile([128, HL * 65], F32R, name=f"V_{t}") for t in range(16)]
    yT_sb = [persist.tile([128, T], F32R, name=f"yT_{i}") for i in range(2)]

    # per-chunk DRAM bounce tensors for the chunked ReduceScatter
    bounce_in = [dram.tile([C, 512], F32, name=f"bounce_in_{n}") for n in range(NT)]
    bounce_rs = [dram.tile([C // 4, 512], F32, name=f"bounce_rs_{n}") for n in range(NT)]

    def q_ap(h):
        return QK_sb[h // 2][(h % 2) * 64:(h % 2) * 64 + 64, :]

    def k_ap(h):
        return QK_sb[2 + h // 2][(h % 2) * 64:(h % 2) * 64 + 64, :]

    # ---- pools ------------------------------------------------------------
    psMM = ctx.enter_context(tc.tile_pool(name="psMM", bufs=2, space="PSUM"))
    psS = ctx.enter_context(tc.tile_pool(name="psS", bufs=2, space="PSUM"))
    psY = ctx.enter_context(tc.tile_pool(name="psY", bufs=2, space="PSUM"))
    pPf = ctx.enter_context(tc.tile_pool(name="pPf", bufs=3))
    pPd = ctx.enter_context(tc.tile_pool(name="pPd", bufs=4))
    pR = ctx.enter_context(tc.tile_pool(name="pR", bufs=2))
    pBC = ctx.enter_context(tc.tile_pool(name="pBC", bufs=2))
    pO = ctx.enter_context(tc.tile_pool(name="pO", bufs=4))

    for n in range(NT):
        nsl = slice(n * 512, (n + 1) * 512)

        # ---- A(n): Q^T / K^T gemm for this chunk (out [ch, tok]) ---------
        for m in range(4):
            ps = psMM.tile([128, 512], F32)
            for kc in range(KC):
                nc.tensor.matmul(
                    ps[:], lhsT=w_qk_sb[kc][:, m * 128:(m + 1) * 128],
                    rhs=xT_sb[kc, n][:], start=(kc == 0), stop=(kc == KC - 1))
            nc.vector.tensor_scalar_add(QK_sb[m][:, nsl], ps[:],
                                        b_qk_sb[:, m:m + 1])

        # ---- B(n): V gemm (out [tok, ch]) + bias + ones column -----------
        for t in range(4):
            ps = psMM.tile([128, 512], F32)
            for kc in range(KC):
                nc.tensor.matmul(
                    ps[:, 0:CL], lhsT=xT_sb[kc, n][:, t * 128:(t + 1) * 128],
                    rhs=w_v_sb[kc][:], start=(kc == 0), stop=(kc == KC - 1))
            tok = n * 4 + t
            vt = V_sb[tok][:].rearrange("p (h c) -> p h c", h=HL)
            nc.vector.tensor_add(
                vt[:, :, 0:64], ps[:, 0:CL].rearrange("p (h c) -> p h c", h=HL),
                b_v_bc[:].rearrange("p (h c) -> p h c", h=HL))
            nc.vector.tensor_copy(vt[:, :, 64:65].rearrange("p h c -> p (h c)"),
                                  ones_sb[:])

        # ---- C(n): causal attention for q-chunk n ------------------------
        nfull = 4 * n
        for h in range(HL):
            y_ps = psY.tile([65, 512], F32)
            # full k-chunks, processed in pairs sharing one [128,1024] S tile
            for pi in range((nfull + 1) // 2):
                j0 = 2 * pi
                two = (j0 + 1 < nfull)
                s = psS.tile([128, 1024], F32)
                nc.tensor.matmul(
                    s[:, 0:512], lhsT=k_ap(h)[:, j0 * 128:(j0 + 1) * 128],
                    rhs=q_ap(h)[:, nsl], start=True, stop=True)
                if two:
                    nc.tensor.matmul(
                        s[:, 512:1024],
                        lhsT=k_ap(h)[:, (j0 + 1) * 128:(j0 + 2) * 128],
                        rhs=q_ap(h)[:, nsl], start=True, stop=True)
                p = pPf.tile([128, 1024], F32R)
                width = 1024 if two else 512
                nc.scalar.activation(p[:, 0:width], s[:, 0:width], Exp,
                                     scale=0.125)
                nc.tensor.matmul(
                    y_ps[:], lhsT=V_sb[j0][:, h * 65:(h + 1) * 65],
                    rhs=p[:, 0:512], start=(pi == 0), stop=False)
                if two:
                    nc.tensor.matmul(
                        y_ps[:], lhsT=V_sb[j0 + 1][:, h * 65:(h + 1) * 65],
                        rhs=p[:, 512:1024], start=False, stop=False)
            # diagonal band: restricted q-window + small triangle masks
            for r in range(4):
                j = nfull + r
                w = min(128 * r, 256)
                s = psS.tile([128, 1024], F32)
                nc.tensor.matmul(
                    s[:, w:512], lhsT=k_ap(h)[:, j * 128:(j + 1) * 128],
                    rhs=q_ap(h)[:, n * 512 + w:(n + 1) * 512],
                    start=True, stop=True)
                p = pPd.tile([128, 512], F32R)
                nc.scalar.activation(p[:, w:512], s[:, w:512], Exp, scale=0.125)
                if r < 3:
                    nc.vector.tensor_mul(p[:, 128 * r:128 * (r + 1)],
                                         p[:, 128 * r:128 * (r + 1)], M_tri[:])
                else:
                    nc.vector.tensor_mul(p[:, 256:512], p[:, 256:512], M_zt[:])
                nc.tensor.matmul(
                    y_ps[:, w:512], lhsT=V_sb[j][:, h * 65:(h + 1) * 65],
                    rhs=p[:, w:512], start=(n == 0 and r == 0), stop=(r == 3))
            # normalize: divide by the folded denominator row
            rec = pR.tile([1, 512], F32R)
            nc.vector.reciprocal(rec[:], y_ps[64:65, :])
            ps = psMM.tile([128, 512], F32)
            nc.tensor.matmul(ps[0:64, :], lhsT=ones_row[:, 0:64], rhs=rec[:],
                             start=True, stop=True)
            bc = pBC.tile([64, 512], F32R)
            nc.vector.tensor_copy(bc[:], ps[0:64, :])
            nc.vector.tensor_mul(
                yT_sb[h // 2][(h % 2) * 64:(h % 2) * 64 + 64, nsl],
                y_ps[0:64, :], bc[:])

        # ---- D(n): output projection for this chunk ----------------------
        for m in range(8):
            ps = psMM.tile([128, 512], F32)
            for kc in range(2):
                nc.tensor.matmul(
                    ps[:], lhsT=w_pr_sb[kc][:, m * 128:(m + 1) * 128],
                    rhs=yT_sb[kc][:, nsl], start=(kc == 0), stop=(kc == 1))
            o_sb = pO.tile([128, 512], F32)
            nc.vector.tensor_scalar_add(o_sb[:], ps[:], b_pr_sb[:, m:m + 1])
            nc.sync.dma_start(bounce_in[n][m * 128:(m + 1) * 128, :], o_sb[:])

        # ---- RS(n): reduce this chunk over the 4-core group --------------
        nc.gpsimd.collective_compute(
            "ReduceScatter", mybir.AluOpType.add, replica_groups=GROUPS,
            ins=[bounce_in[n][:]], outs=[bounce_rs[n][:]])
        nc.sync.dma_start(out_rs[:, nsl], bounce_rs[n][:])


_NC_CACHE = None


def _get_nc():
    global _NC_CACHE
    if _NC_CACHE is None:
        _NC_CACHE = _build_bass()
    return _NC_CACHE


def kernel(x, w_qkv, b_qkv, w_proj, b_proj, **_kw):
    x = np.asarray(x, dtype=np.float32)
    w_qkv = np.asarray(w_qkv, dtype=np.float32)
    b_qkv = np.asarray(b_qkv, dtype=np.float32)
    w_proj = np.asarray(w_proj, dtype=np.float32)
    b_proj = np.asarray(b_proj, dtype=np.float32)

    nc = _get_nc()
    in_maps = []
    for c in range(N_CORES):
        b = c // 4
        g = c % 4
        qs = slice(g * CL, (g + 1) * CL)
        ks = slice(C + g * CL, C + (g + 1) * CL)
        vs = slice(2 * C + g * CL, 2 * C + (g + 1) * CL)
        in_maps.append({
            "xT": np.ascontiguousarray(x[b].T),
            "w_qk": np.ascontiguousarray(
                np.concatenate([w_qkv[:, qs], w_qkv[:, ks]], axis=1)),
            "b_qk": np.ascontiguousarray(
                np.concatenate([b_qkv[qs], b_qkv[ks]])),
            "w_v": np.ascontiguousarray(w_qkv[:, vs]),
            "b_v": np.ascontiguousarray(b_qkv[vs]),
            "w_pr": np.ascontiguousarray(w_proj[g * CL:(g + 1) * CL, :]),
            "b_pr": b_proj if g == 0 else np.zeros_like(b_proj),
        })

    global _last_in_maps
    _last_in_maps = in_maps
    res = bass_utils.run_bass_kernel_spmd(nc, in_maps, core_ids=list(range(N_CORES)))

    out = np.empty((B, T, C), dtype=np.float32)
    for b in range(B):
        projT = np.concatenate(
            [res.results[4 * b + r]["out_rs"] for r in range(4)], axis=0)
        out[b] = projT.T
    return out


if __name__ == "__main__":
    rng = np.random.RandomState(0)
    ins = {
        "x": rng.randn(B, T, C).astype(np.float32),
        "w_qkv": rng.randn(C, 3 * C).astype(np.float32) / 32,
        "b_qkv": rng.randn(3 * C).astype(np.float32) / 32,
        "w_proj": rng.randn(C, C).astype(np.float32) / 32,
        "b_proj": rng.randn(C).astype(np.float32) / 32,
    }
    y = kernel(**ins)
    print("kernel ran, out shape", y.shape)


# revision 6
# speedup vs baseline: 1.2383x; 1.0018x over previous
"""Causal self-attention (B=2, T=2048, C=1024, H=16) on 8 trn2 NeuronCores.

Sharding: data-parallel over batch (2 groups of 4 cores) x tensor-parallel over
heads (4 heads / core). Per-token-chunk pipeline: for each 512-token chunk the
core computes Q^T/K^T ([ch,tok]) and V ([tok,ch]) slices, causal attention for
its 4 heads (softmax denominator folded into the PV matmul via a ones column),
its slice of the output projection, and a chunked ReduceScatter over the 4-core
group that overlaps the next chunk's compute. Diagonal attention blocks are
computed over a restricted q-window with the triangle masked by a small DVE
multiply, so no full-width mask multiplies and no wasted exp columns.
"""

import sys

for _p in ("/opt/trn_rl_repo",):
    if _p not in sys.path:
        sys.path.append(_p)

import numpy as np
from contextlib import ExitStack

import concourse.bass as bass
import concourse.mybir as mybir
import concourse.tile as tile
from concourse import bass_utils

B, T, C, H = 2, 2048, 1024, 16
D = C // H              # 64
N_CORES = 8
GROUPS = [[0, 1, 2, 3], [4, 5, 6, 7]]
HL = 4                  # heads per core
CL = HL * D             # 256 local channels
KC = C // 128           # 8 contraction chunks of 128
NT = T // 512           # 4 token chunks of 512
F32 = mybir.dt.float32
F32R = mybir.dt.float32r


def _legalize_waits(nc):
    """This walrus build allows at most ONE sync-wait per instruction. Move
    extra waits onto same-engine NoOps inserted just before the instruction."""
    n_split = 0
    for f in nc.m.functions:
        for bb in f.blocks:
            out = []
            for inst in bb.instructions:
                si = inst.sync_info
                waits = list(si.on_wait) if si is not None and si.on_wait else []
                if len(waits) > 1:
                    for i, w in enumerate(waits[:-1]):
                        out.append(
                            mybir.InstNoOp(
                                name=f"wsplit_{inst.name}_{i}",
                                engine=inst.engine,
                                ins=[],
                                outs=[],
                                sync_info=mybir.SyncInfo(on_wait=[w], on_update=[]),
                            )
                        )
                        n_split += 1
                    si.on_wait = [waits[-1]]
                out.append(inst)
            bb.instructions = out
    return n_split


def _build_bass():
    nc = bass.Bass("TRN2", target_bir_lowering=False, debug=False,
                   num_devices=N_CORES)

    xT = nc.dram_tensor("xT", [C, T], F32R, kind="ExternalInput").ap()
    w_qk = nc.dram_tensor("w_qk", [C, 2 * CL], F32R, kind="ExternalInput").ap()
    b_qk = nc.dram_tensor("b_qk", [2 * CL], F32, kind="ExternalInput").ap()
    w_v = nc.dram_tensor("w_v", [C, CL], F32R, kind="ExternalInput").ap()
    b_v = nc.dram_tensor("b_v", [CL], F32R, kind="ExternalInput").ap()
    w_pr = nc.dram_tensor("w_pr", [CL, C], F32R, kind="ExternalInput").ap()
    b_pr = nc.dram_tensor("b_pr", [C], F32, kind="ExternalInput").ap()
    out_rs = nc.dram_tensor("out_rs", [C // 4, T], F32, kind="ExternalOutput").ap()

    with tile.TileContext(nc) as tc:
        with ExitStack() as ctx:
            with nc.allow_low_precision(reason="float32r is 4-byte; full fp32 accumulate"):
                _build_body(ctx, tc, nc, xT, w_qk, b_qk, w_v, b_v, w_pr, b_pr, out_rs)

    _legalize_waits(nc)
    return nc


def _build_body(ctx, tc, nc, xT, w_qk, b_qk, w_v, b_v, w_pr, b_pr, out_rs):
    Exp = mybir.ActivationFunctionType.Exp

    persist = ctx.enter_context(tc.tile_pool(name="persist", bufs=1))
    dram = ctx.enter_context(tc.tile_pool(name="dram", bufs=1, space="DRAM"))

    # ---- constant / bias tiles -------------------------------------------
    b_qk_sb = persist.tile([128, 4], F32, name="b_qk_sb")
    nc.sync.dma_start(b_qk_sb[:], b_qk.rearrange("(m p) -> p m", p=128))
    b_pr_sb = persist.tile([128, 8], F32, name="b_pr_sb")
    nc.sync.dma_start(b_pr_sb[:], b_pr.rearrange("(m p) -> p m", p=128))
    b_v_row = persist.tile([1, CL], F32R, name="b_v_row")
    nc.sync.dma_start(b_v_row[:], b_v.rearrange("(a c) -> a c", a=1))
    ones_sb = persist.tile([128, HL], F32, name="ones_sb")
    nc.gpsimd.memset(ones_sb[:], 1.0)
    ones_f32 = persist.tile([1, 128], F32, name="ones_f32")
    nc.gpsimd.memset(ones_f32[:], 1.0)
    ones_row = persist.tile([1, 128], F32R, name="ones_row")
    nc.vector.tensor_copy(ones_row[:], ones_f32[:])
    # broadcast b_v across partitions via a rank-1 matmul
    b_v_bc = persist.tile([128, CL], F32, name="b_v_bc")
    with tc.tile_pool(name="psInit", bufs=1, space="PSUM") as psI:
        bvp = psI.tile([128, CL], F32)
        nc.tensor.matmul(bvp[:], lhsT=ones_row[:], rhs=b_v_row[:],
                         start=True, stop=True)
        nc.vector.tensor_copy(b_v_bc[:], bvp[:])
    # 0/1 masks for diagonal blocks:
    #   M_tri[x, y] = y - x >= 0            (128x128 lower-left triangle)
    #   M_zt[x, y]  = y - x - 128 >= 0      (zeros strip | triangle, 128x256)
    M_tri = persist.tile([128, 128], F32, name="M_tri")
    nc.gpsimd.memset(M_tri[:], 1.0)
    nc.gpsimd.affine_select(
        out=M_tri[:], in_=M_tri[:], compare_op=mybir.AluOpType.is_ge, fill=0.0,
        base=0, pattern=[[1, 128]], channel_multiplier=-1)
    M_zt = persist.tile([128, 256], F32, name="M_zt")
    nc.gpsimd.memset(M_zt[:], 1.0)
    nc.gpsimd.affine_select(
        out=M_zt[:], in_=M_zt[:], compare_op=mybir.AluOpType.is_ge, fill=0.0,
        base=-128, pattern=[[1, 256]], channel_multiplier=-1)

    # ---- weight + activation loads (xT chunk 0 first so PE starts early) --
    xT_sb = {}

    def load_x_chunk(n):
        for kc in range(KC):
            t = persist.tile([128, 512], F32R, name=f"xT_{kc}_{n}")
            nc.sync.dma_start(
                t[:], xT[kc * 128:(kc + 1) * 128, n * 512:(n + 1) * 512])
            xT_sb[kc, n] = t

    load_x_chunk(0)
    w_qk_sb = []
    for kc in range(KC):
        t = persist.tile([128, 2 * CL], F32R, name=f"w_qk_{kc}")
        nc.sync.dma_start(t[:], w_qk[kc * 128:(kc + 1) * 128, :])
        w_qk_sb.append(t)
    w_v_sb = []
    for kc in range(KC):
        t = persist.tile([128, CL], F32R, name=f"w_v_{kc}")
        nc.sync.dma_start(t[:], w_v[kc * 128:(kc + 1) * 128, :])
        w_v_sb.append(t)
    w_pr_sb = []
    for kc in range(2):
        t = persist.tile([128, C], F32R, name=f"w_pr_{kc}")
        nc.sync.dma_start(t[:], w_pr[kc * 128:(kc + 1) * 128, :])
        w_pr_sb.append(t)
    for n in range(1, NT):
        load_x_chunk(n)

    # ---- persistent intermediates ----------------------------------------
    # QK_sb[m]: m=0,1 -> Q channels (heads 0,1 | 2,3), m=2,3 -> K channels
    QK_sb = [persist.tile([128, T], F32R, name=f"QK_{m}") for m in range(4)]
    # V in [tok, ch] layout, 65 cols/head: col h*65+64 is the ones column
    V_sb = [persist.tile([128, HL * 65], F32R, name=f"V_{t}") for t in range(16)]
    yT_sb = [persist.tile([128, T], F32R, name=f"yT_{i}") for i in range(2)]

    # per-chunk DRAM bounce tensors for the chunked ReduceScatter
    bounce_in = [dram.tile([C, 512], F32, name=f"bounce_in_{n}") for n in range(NT)]
    bounce_rs = [dram.tile([C // 4, 512], F32, name=f"bounce_rs_{n}") for n in range(NT)]

    def q_ap(h):
        return QK_sb[h // 2][(h % 2) * 64:(h % 2) * 64 + 64, :]

    def k_ap(h):
        return QK_sb[2 + h // 2][(h % 2) * 64:(h % 2) * 64 + 64, :]

    # ---- pools ------------------------------------------------------------
    psMM = ctx.enter_context(tc.tile_pool(name="psMM", bufs=2, space="PSUM"))
    psS = ctx.enter_context(tc.tile_pool(name="psS", bufs=2, space="PSUM"))
    psY = ctx.enter_context(tc.tile_pool(name="psY", bufs=2, space="PSUM"))
    pPf = ctx.enter_context(tc.tile_pool(name="pPf", bufs=3))
    pPd = ctx.enter_context(tc.tile_pool(name="pPd", bufs=4))
    pR = ctx.enter_context(tc.tile_pool(name="pR", bufs=2))
    pBC = ctx.enter_context(tc.tile_pool(name="pBC", bufs=2))
    pO = ctx.enter_context(tc.tile_pool(name="pO", bufs=4))

    for n in range(NT):
        nsl = slice(n * 512, (n + 1) * 512)

        # ---- A(n): Q^T / K^T gemm for this chunk (out [ch, tok]) ---------
        for m in range(4):
            ps = psMM.tile([128, 512], F32)
            for kc in range(KC):
                nc.tensor.matmul(
                    ps[:], lhsT=w_qk_sb[kc][:, m * 128:(m + 1) * 128],
                    rhs=xT_sb[kc, n][:], start=(kc == 0), stop=(kc == KC - 1))
            nc.vector.tensor_scalar_add(QK_sb[m][:, nsl], ps[:],
                                        b_qk_sb[:, m:m + 1])

        # ---- B(n): V gemm (out [tok, ch]) + bias + ones column -----------
        for t in range(4):
            ps = psMM.tile([128, 512], F32)
            for kc in range(KC):
                nc.tensor.matmul(
                    ps[:, 0:CL], lhsT=xT_sb[kc, n][:, t * 128:(t + 1) * 128],
                    rhs=w_v_sb[kc][:], start=(kc == 0), stop=(kc == KC - 1))
            tok = n * 4 + t
            vt = V_sb[tok][:].rearrange("p (h c) -> p h c", h=HL)
            nc.vector.tensor_add(
                vt[:, :, 0:64], ps[:, 0:CL].rearrange("p (h c) -> p h c", h=HL),
                b_v_bc[:].rearrange("p (h c) -> p h c", h=HL))
            nc.vector.tensor_copy(vt[:, :, 64:65].rearrange("p h c -> p (h c)"),
                                  ones_sb[:])

        # ---- C(n): causal attention for q-chunk n ------------------------
        # The per-head normalize tail (recip -> broadcast matmul -> mul) is
        # emitted a couple of S-matmuls INTO the next head so the PE never
        # stalls waiting for the DVE reciprocal.
        nfull = 4 * n
        pending_tail = [None]

        def emit_tail(h, y_ps):
            rec = pR.tile([1, 512], F32R)
            nc.vector.reciprocal(rec[:], y_ps[64:65, :])
            ps = psMM.tile([128, 512], F32)
            nc.tensor.matmul(ps[0:64, :], lhsT=ones_row[:, 0:64], rhs=rec[:],
                             start=True, stop=True)
            bc = pBC.tile([64, 512], F32R)
            nc.vector.tensor_copy(bc[:], ps[0:64, :])
            nc.vector.tensor_mul(
                yT_sb[h // 2][(h % 2) * 64:(h % 2) * 64 + 64, nsl],
                y_ps[0:64, :], bc[:])

        for h in range(HL):
            y_ps = psY.tile([65, 512], F32)
            s_seen = 0

            def after_s(pending_tail=pending_tail):
                nonlocal s_seen
                s_seen += 1
                if s_seen == 2 and pending_tail[0] is not None:
                    pending_tail[0]()
                    pending_tail[0] = None

            # full k-chunks, processed in pairs sharing one [128,1024] S tile
            for pi in range(nfull // 2):
                j0 = 2 * pi
                s = psS.tile([128, 1024], F32)
                nc.tensor.matmul(
                    s[:, 0:512], lhsT=k_ap(h)[:, j0 * 128:(j0 + 1) * 128],
                    rhs=q_ap(h)[:, nsl], start=True, stop=True)
                after_s()
                nc.tensor.matmul(
                    s[:, 512:1024],
                    lhsT=k_ap(h)[:, (j0 + 1) * 128:(j0 + 2) * 128],
                    rhs=q_ap(h)[:, nsl], start=True, stop=True)
                after_s()
                p = pPf.tile([128, 1024], F32R)
                nc.scalar.activation(p[:], s[:], Exp, scale=0.125)
                nc.tensor.matmul(
                    y_ps[:], lhsT=V_sb[j0][:, h * 65:(h + 1) * 65],
                    rhs=p[:, 0:512], start=(pi == 0), stop=False)
                nc.tensor.matmul(
                    y_ps[:], lhsT=V_sb[j0 + 1][:, h * 65:(h + 1) * 65],
                    rhs=p[:, 512:1024], start=False, stop=False)
            # diagonal band: restricted q-window + small triangle masks
            for r in range(4):
                j = nfull + r
                w = min(128 * r, 256)
                s = psS.tile([128, 1024], F32)
                nc.tensor.matmul(
                    s[:, w:512], lhsT=k_ap(h)[:, j * 128:(j + 1) * 128],
                    rhs=q_ap(h)[:, n * 512 + w:(n + 1) * 512],
                    start=True, stop=True)
                after_s()
                p = pPd.tile([128, 512], F32R)
                nc.scalar.activation(p[:, w:512], s[:, w:512], Exp, scale=0.125)
                if r < 3:
                    nc.vector.tensor_mul(p[:, 128 * r:128 * (r + 1)],
                                         p[:, 128 * r:128 * (r + 1)], M_tri[:])
                else:
                    nc.vector.tensor_mul(p[:, 256:512], p[:, 256:512], M_zt[:])
                nc.tensor.matmul(
                    y_ps[:, w:512], lhsT=V_sb[j][:, h * 65:(h + 1) * 65],
                    rhs=p[:, w:512], start=(n == 0 and r == 0), stop=(r == 3))
            pending_tail[0] = (lambda h=h, y_ps=y_ps: emit_tail(h, y_ps))
        pending_tail[0]()

        # ---- D(n): output projection for this chunk ----------------------
        for m in range(8):
            ps = psMM.tile([128, 512], F32)
            for kc in range(2):
                nc.tensor.matmul(
                    ps[:], lhsT=w_pr_sb[kc][:, m * 128:(m + 1) * 128],
                    rhs=yT_sb[kc][:, nsl], start=(kc == 0), stop=(kc == 1))
            o_sb = pO.tile([128, 512], F32)
            if m % 2 == 0:
                nc.scalar.activation(o_sb[:], ps[:],
                                     mybir.ActivationFunctionType.Identity,
                                     bias=b_pr_sb[:, m:m + 1])
            else:
                nc.vector.tensor_scalar_add(o_sb[:], ps[:], b_pr_sb[:, m:m + 1])
            nc.sync.dma_start(bounce_in[n][m * 128:(m + 1) * 128, :], o_sb[:])

        # ---- RS(n): reduce this chunk over the 4-core group --------------
        nc.gpsimd.collective_compute(
            "ReduceScatter", mybir.AluOpType.add, replica_groups=GROUPS,
            ins=[bounce_in[n][:]], outs=[bounce_rs[n][:]])
        nc.sync.dma_start(out_rs[:, nsl], bounce_rs[n][:])


_NC_CACHE = None


def _get_nc():
    global _NC_CACHE
    if _NC_CACHE is None:
        _NC_CACHE = _build_bass()
    return _NC_CACHE


def kernel(x, w_qkv, b_qkv, w_proj, b_proj, **_kw):
    x = np.asarray(x, dtype=np.float32)
    w_qkv = np.asarray(w_qkv, dtype=np.float32)
    b_qkv = np.asarray(b_qkv, dtype=np.float32)
    w_proj = np.asarray(w_proj, dtype=np.float32)
    b_proj = np.asarray(b_proj, dtype=np.float32)

    nc = _get_nc()
    in_maps = []
    for c in range(N_CORES):
        b = c // 4
        g = c % 4
        qs = slice(g * CL, (g + 1) * CL)
        ks = slice(C + g * CL, C + (g + 1) * CL)
        vs = slice(2 * C + g * CL, 2 * C + (g + 1) * CL)
        in_maps.append({
            "xT": np.ascontiguousarray(x[b].T),
            "w_qk": np.ascontiguousarray(
                np.concatenate([w_qkv[:, qs], w_qkv[:, ks]], axis=1)),
            "b_qk": np.ascontiguousarray(
                np.concatenate([b_qkv[qs], b_qkv[ks]])),
            "w_v": np.ascontiguousarray(w_qkv[:, vs]),
            "b_v": np.ascontiguousarray(b_qkv[vs]),
            "w_pr": np.ascontiguousarray(w_proj[g * CL:(g + 1) * CL, :]),
            "b_pr": b_proj if g == 0 else np.zeros_like(b_proj),
        })

    global _last_in_maps
    _last_in_maps = in_maps
    res = bass_utils.run_bass_kernel_spmd(nc, in_maps, core_ids=list(range(N_CORES)))

    out = np.empty((B, T, C), dtype=np.float32)
    for b in range(B):
        projT = np.concatenate(
            [res.results[4 * b + r]["out_rs"] for r in range(4)], axis=0)
        out[b] = projT.T
    return out


if __name__ == "__main__":
    rng = np.random.RandomState(0)
    ins = {
        "x": rng.randn(B, T, C).astype(np.float32),
        "w_qkv": rng.randn(C, 3 * C).astype(np.float32) / 32,
        "b_qkv": rng.randn(3 * C).astype(np.float32) / 32,
        "w_proj": rng.randn(C, C).astype(np.float32) / 32,
        "b_proj": rng.randn(C).astype(np.float32) / 32,
    }
    y = kernel(**ins)
    print("kernel ran, out shape", y.shape)


# revision 11
# speedup vs baseline: 1.2680x; 1.0240x over previous
"""Causal self-attention (B=2, T=2048, C=1024, H=16) on 8 trn2 NeuronCores.

Sharding: data-parallel over batch (2 groups of 4 cores) x tensor-parallel over
heads (4 heads / core). Per-token-chunk pipeline: for each 512-token chunk the
core computes Q^T/K^T ([ch,tok]) and V ([tok,ch]) slices, causal attention for
its 4 heads (softmax denominator folded into the PV matmul via a ones column),
its slice of the output projection, and a chunked ReduceScatter over the 4-core
group that overlaps the next chunk's compute. Diagonal attention blocks are
computed over a restricted q-window with the triangle masked by a small DVE
multiply, so no full-width mask multiplies and no wasted exp columns.
"""

import sys

for _p in ("/opt/trn_rl_repo",):
    if _p not in sys.path:
        sys.path.append(_p)

import numpy as np
from contextlib import ExitStack

import concourse.bass as bass
import concourse.mybir as mybir
import concourse.tile as tile
from concourse import bass_utils

B, T, C, H = 2, 2048, 1024, 16
D = C // H              # 64
N_CORES = 8
GROUPS = [[0, 1, 2, 3], [4, 5, 6, 7]]
HL = 4                  # heads per core
CL = HL * D             # 256 local channels
KC = C // 128           # 8 contraction chunks of 128
NT = T // 512           # 4 token chunks of 512
F32 = mybir.dt.float32
F32R = mybir.dt.float32r


def _legalize_waits(nc):
    """This walrus build allows at most ONE sync-wait per instruction. Move
    extra waits onto same-engine NoOps inserted just before the instruction."""
    n_split = 0
    for f in nc.m.functions:
        for bb in f.blocks:
            out = []
            for inst in bb.instructions:
                si = inst.sync_info
                waits = list(si.on_wait) if si is not None and si.on_wait else []
                if len(waits) > 1:
                    for i, w in enumerate(waits[:-1]):
                        out.append(
                            mybir.InstNoOp(
                                name=f"wsplit_{inst.name}_{i}",
                                engine=inst.engine,
                                ins=[],
                                outs=[],
                                sync_info=mybir.SyncInfo(on_wait=[w], on_update=[]),
                            )
                        )
                        n_split += 1
                    si.on_wait = [waits[-1]]
                out.append(inst)
            bb.instructions = out
    return n_split


def _build_bass():
    nc = bass.Bass("TRN2", target_bir_lowering=False, debug=False,
                   num_devices=N_CORES)

    xT = nc.dram_tensor("xT", [C, T], F32R, kind="ExternalInput").ap()
    w_qk = nc.dram_tensor("w_qk", [C, 2 * CL], F32R, kind="ExternalInput").ap()
    b_qk = nc.dram_tensor("b_qk", [2 * CL], F32, kind="ExternalInput").ap()
    w_v = nc.dram_tensor("w_v", [C, CL], F32R, kind="ExternalInput").ap()
    b_v = nc.dram_tensor("b_v", [CL], F32R, kind="ExternalInput").ap()
    w_pr = nc.dram_tensor("w_pr", [CL, C], F32R, kind="ExternalInput").ap()
    b_pr = nc.dram_tensor("b_pr", [C], F32, kind="ExternalInput").ap()
    out_rs = nc.dram_tensor("out_rs", [C // 4, T], F32, kind="ExternalOutput").ap()

    with tile.TileContext(nc) as tc:
        with ExitStack() as ctx:
            with nc.allow_low_precision(reason="float32r is 4-byte; full fp32 accumulate"):
                _build_body(ctx, tc, nc, xT, w_qk, b_qk, w_v, b_v, w_pr, b_pr, out_rs)

    _legalize_waits(nc)
    return nc


def _build_body(ctx, tc, nc, xT, w_qk, b_qk, w_v, b_v, w_pr, b_pr, out_rs):
    Exp = mybir.ActivationFunctionType.Exp

    persist = ctx.enter_context(tc.tile_pool(name="persist", bufs=1))
    dram = ctx.enter_context(tc.tile_pool(name="dram", bufs=1, space="DRAM"))

    # ---- constant / bias tiles -------------------------------------------
    b_qk_sb = persist.tile([128, 4], F32, name="b_qk_sb")
    nc.sync.dma_start(b_qk_sb[:], b_qk.rearrange("(m p) -> p m", p=128))
    b_pr_sb = persist.tile([128, 8], F32, name="b_pr_sb")
    nc.sync.dma_start(b_pr_sb[:], b_pr.rearrange("(m p) -> p m", p=128))
    b_v_row = persist.tile([1, CL], F32R, name="b_v_row")
    nc.sync.dma_start(b_v_row[:], b_v.rearrange("(a c) -> a c", a=1))
    ones_sb = persist.tile([128, HL], F32, name="ones_sb")
    nc.gpsimd.memset(ones_sb[:], 1.0)
    ones_f32 = persist.tile([1, 128], F32, name="ones_f32")
    nc.gpsimd.memset(ones_f32[:], 1.0)
    ones_row = persist.tile([1, 128], F32R, name="ones_row")
    nc.vector.tensor_copy(ones_row[:], ones_f32[:])
    # broadcast b_v across partitions via a rank-1 matmul
    b_v_bc = persist.tile([128, CL], F32, name="b_v_bc")
    with tc.tile_pool(name="psInit", bufs=1, space="PSUM") as psI:
        bvp = psI.tile([128, CL], F32)
        nc.tensor.matmul(bvp[:], lhsT=ones_row[:], rhs=b_v_row[:],
                         start=True, stop=True)
        nc.vector.tensor_copy(b_v_bc[:], bvp[:])
    # 0/1 masks for diagonal blocks:
    #   M_tri[x, y] = y - x >= 0            (128x128 lower-left triangle)
    #   M_zt[x, y]  = y - x - 128 >= 0      (zeros strip | triangle, 128x256)
    M_tri = persist.tile([128, 128], F32, name="M_tri")
    nc.gpsimd.memset(M_tri[:], 1.0)
    nc.gpsimd.affine_select(
        out=M_tri[:], in_=M_tri[:], compare_op=mybir.AluOpType.is_ge, fill=0.0,
        base=0, pattern=[[1, 128]], channel_multiplier=-1)
    M_zt = persist.tile([128, 256], F32, name="M_zt")
    nc.gpsimd.memset(M_zt[:], 1.0)
    nc.gpsimd.affine_select(
        out=M_zt[:], in_=M_zt[:], compare_op=mybir.AluOpType.is_ge, fill=0.0,
        base=-128, pattern=[[1, 256]], channel_multiplier=-1)

    # ---- weight + activation loads (xT chunk 0 first so PE starts early) --
    # x chunks rotate through a 16-slot pool (2 chunks resident) with
    # prefetch, instead of pinning all 64KB/partition of xT in SBUF.
    pX = ctx.enter_context(tc.tile_pool(name="pX", bufs=16))
    xT_sb = {}

    def load_x_chunk(n):
        for kc in range(KC):
            xt = pX.tile([128, 512], F32R)
            nc.sync.dma_start(
                xt[:], xT[kc * 128:(kc + 1) * 128, n * 512:(n + 1) * 512])
            xT_sb[kc, n] = xt

    load_x_chunk(0)
    w_qk_sb = []
    for kc in range(KC):
        t = persist.tile([128, 2 * CL], F32R, name=f"w_qk_{kc}")
        nc.sync.dma_start(t[:], w_qk[kc * 128:(kc + 1) * 128, :])
        w_qk_sb.append(t)
    w_v_sb = []
    for kc in range(KC):
        t = persist.tile([128, CL], F32R, name=f"w_v_{kc}")
        nc.sync.dma_start(t[:], w_v[kc * 128:(kc + 1) * 128, :])
        w_v_sb.append(t)
    w_pr_sb = []
    for kc in range(2):
        t = persist.tile([128, C], F32R, name=f"w_pr_{kc}")
        nc.sync.dma_start(t[:], w_pr[kc * 128:(kc + 1) * 128, :])
        w_pr_sb.append(t)
    load_x_chunk(1)

    # ---- persistent intermediates ----------------------------------------
    # QK_sb[m]: m=0,1 -> Q channels (heads 0,1 | 2,3), m=2,3 -> K channels
    QK_sb = [persist.tile([128, T], F32R, name=f"QK_{m}") for m in range(4)]
    # V in [tok, ch] layout, 65 cols/head: col h*65+64 is the ones column
    V_sb = [persist.tile([128, HL * 65], F32R, name=f"V_{t}") for t in range(16)]
    yT_sb = [persist.tile([128, T], F32R, name=f"yT_{i}") for i in range(2)]

    # per-chunk DRAM bounce tensors for the chunked ReduceScatter
    bounce_in = [dram.tile([C, 512], F32, name=f"bounce_in_{n}") for n in range(NT)]
    bounce_rs = [dram.tile([C // 4, 512], F32, name=f"bounce_rs_{n}") for n in range(NT)]

    def q_ap(h):
        return QK_sb[h // 2][(h % 2) * 64:(h % 2) * 64 + 64, :]

    def k_ap(h):
        return QK_sb[2 + h // 2][(h % 2) * 64:(h % 2) * 64 + 64, :]

    # ---- pools ------------------------------------------------------------
    psMM = ctx.enter_context(tc.tile_pool(name="psMM", bufs=2, space="PSUM"))
    psS = ctx.enter_context(tc.tile_pool(name="psS", bufs=2, space="PSUM"))
    psY = ctx.enter_context(tc.tile_pool(name="psY", bufs=2, space="PSUM"))
    pPf = ctx.enter_context(tc.tile_pool(name="pPf", bufs=7))
    pPd = ctx.enter_context(tc.tile_pool(name="pPd", bufs=5))
    pR = ctx.enter_context(tc.tile_pool(name="pR", bufs=2))
    pBC = ctx.enter_context(tc.tile_pool(name="pBC", bufs=2))
    pO = ctx.enter_context(tc.tile_pool(name="pO", bufs=4))

    for n in range(NT):
        nsl = slice(n * 512, (n + 1) * 512)
        if n + 2 < NT:
            load_x_chunk(n + 2)

        # ---- A(n): Q^T / K^T gemm for this chunk (out [ch, tok]) ---------
        for m in range(4):
            ps = psMM.tile([128, 512], F32)
            for kc in range(KC):
                nc.tensor.matmul(
                    ps[:], lhsT=w_qk_sb[kc][:, m * 128:(m + 1) * 128],
                    rhs=xT_sb[kc, n][:], start=(kc == 0), stop=(kc == KC - 1))
            nc.vector.tensor_scalar_add(QK_sb[m][:, nsl], ps[:],
                                        b_qk_sb[:, m:m + 1])

        # ---- B(n): V gemm (out [tok, ch]) + bias + ones column -----------
        for t in range(4):
            ps = psMM.tile([128, 512], F32)
            for kc in range(KC):
                nc.tensor.matmul(
                    ps[:, 0:CL], lhsT=xT_sb[kc, n][:, t * 128:(t + 1) * 128],
                    rhs=w_v_sb[kc][:], start=(kc == 0), stop=(kc == KC - 1))
            tok = n * 4 + t
            vt = V_sb[tok][:].rearrange("p (h c) -> p h c", h=HL)
            nc.vector.tensor_add(
                vt[:, :, 0:64], ps[:, 0:CL].rearrange("p (h c) -> p h c", h=HL),
                b_v_bc[:].rearrange("p (h c) -> p h c", h=HL))
            nc.vector.tensor_copy(vt[:, :, 64:65].rearrange("p h c -> p (h c)"),
                                  ones_sb[:])

        # ---- C(n): causal attention for q-chunk n ------------------------
        # The per-head normalize tail (recip -> broadcast matmul -> mul) is
        # emitted a couple of S-matmuls INTO the next head so the PE never
        # stalls waiting for the DVE reciprocal.
        nfull = 4 * n
        pending_tail = [None]

        def emit_tail(h, y_ps):
            rec = pR.tile([1, 512], F32R)
            nc.vector.reciprocal(rec[:], y_ps[64:65, :])
            ps = psMM.tile([128, 512], F32)
            nc.tensor.matmul(ps[0:64, :], lhsT=ones_row[:, 0:64], rhs=rec[:],
                             start=True, stop=True)
            bc = pBC.tile([64, 512], F32R)
            nc.vector.tensor_copy(bc[:], ps[0:64, :])
            nc.vector.tensor_mul(
                yT_sb[h // 2][(h % 2) * 64:(h % 2) * 64 + 64, nsl],
                y_ps[0:64, :], bc[:])

        for h in range(HL):
            y_ps = psY.tile([65, 512], F32)
            s_seen = 0

            def after_s(pending_tail=pending_tail):
                nonlocal s_seen
                s_seen += 1
                if s_seen == 2 and pending_tail[0] is not None:
                    pending_tail[0]()
                    pending_tail[0] = None

            # pass 1: stream ALL S matmuls for this head while Act runs the
            # exps into a deep p-tile buffer (keeps PE ahead of Act).
            p_full, p_diag = [], []
            for pi in range(nfull // 2):
                j0 = 2 * pi
                s = psS.tile([128, 1024], F32)
                nc.tensor.matmul(
                    s[:, 0:512], lhsT=k_ap(h)[:, j0 * 128:(j0 + 1) * 128],
                    rhs=q_ap(h)[:, nsl], start=True, stop=True)
                after_s()
                nc.tensor.matmul(
                    s[:, 512:1024],
                    lhsT=k_ap(h)[:, (j0 + 1) * 128:(j0 + 2) * 128],
                    rhs=q_ap(h)[:, nsl], start=True, stop=True)
                after_s()
                p = pPf.tile([128, 1024], F32R)
                nc.scalar.activation(p[:], s[:], Exp, scale=0.125)
                p_full.append(p)
            for r in range(4):
                j = nfull + r
                w = min(128 * r, 256)
                s = psS.tile([128, 1024], F32)
                nc.tensor.matmul(
                    s[:, w:512], lhsT=k_ap(h)[:, j * 128:(j + 1) * 128],
                    rhs=q_ap(h)[:, n * 512 + w:(n + 1) * 512],
                    start=True, stop=True)
                after_s()
                p = pPd.tile([128, 512], F32R)
                nc.scalar.activation(p[:, w:512], s[:, w:512], Exp, scale=0.125)
                if r < 3:
                    nc.vector.tensor_mul(p[:, 128 * r:128 * (r + 1)],
                                         p[:, 128 * r:128 * (r + 1)], M_tri[:])
                else:
                    nc.vector.tensor_mul(p[:, 256:512], p[:, 256:512], M_zt[:])
                p_diag.append(p)
            # pass 2: PV matmuls consume the buffered p tiles in order
            for pi in range(nfull // 2):
                j0 = 2 * pi
                p = p_full[pi]
                nc.tensor.matmul(
                    y_ps[:], lhsT=V_sb[j0][:, h * 65:(h + 1) * 65],
                    rhs=p[:, 0:512], start=(pi == 0), stop=False)
                nc.tensor.matmul(
                    y_ps[:], lhsT=V_sb[j0 + 1][:, h * 65:(h + 1) * 65],
                    rhs=p[:, 512:1024], start=False, stop=False)
            for r in range(4):
                j = nfull + r
                w = min(128 * r, 256)
                p = p_diag[r]
                nc.tensor.matmul(
                    y_ps[:, w:512], lhsT=V_sb[j][:, h * 65:(h + 1) * 65],
                    rhs=p[:, w:512], start=(n == 0 and r == 0), stop=(r == 3))
            pending_tail[0] = (lambda h=h, y_ps=y_ps: emit_tail(h, y_ps))
        pending_tail[0]()

        # ---- D(n): output projection for this chunk ----------------------
        for m in range(8):
            ps = psMM.tile([128, 512], F32)
            for kc in range(2):
                nc.tensor.matmul(
                    ps[:], lhsT=w_pr_sb[kc][:, m * 128:(m + 1) * 128],
                    rhs=yT_sb[kc][:, nsl], start=(kc == 0), stop=(kc == 1))
            o_sb = pO.tile([128, 512], F32)
            if m % 2 == 0:
                nc.scalar.activation(o_sb[:], ps[:],
                                     mybir.ActivationFunctionType.Identity,
                                     bias=b_pr_sb[:, m:m + 1])
            else:
                nc.vector.tensor_scalar_add(o_sb[:], ps[:], b_pr_sb[:, m:m + 1])
            nc.sync.dma_start(bounce_in[n][m * 128:(m + 1) * 128, :], o_sb[:])

        # ---- RS(n): reduce this chunk over the 4-core group --------------
        nc.gpsimd.collective_compute(
            "ReduceScatter", mybir.AluOpType.add, replica_groups=GROUPS,
            ins=[bounce_in[n][:]], outs=[bounce_rs[n][:]])
        nc.sync.dma_start(out_rs[:, nsl], bounce_rs[n][:])


_NC_CACHE = None


def _get_nc():
    global _NC_CACHE
    if _NC_CACHE is None:
        _NC_CACHE = _build_bass()
    return _NC_CACHE


def kernel(x, w_qkv, b_qkv, w_proj, b_proj, **_kw):
    x = np.asarray(x, dtype=np.float32)
    w_qkv = np.asarray(w_qkv, dtype=np.float32)
    b_qkv = np.asarray(b_qkv, dtype=np.float32)
    w_proj = np.asarray(w_proj, dtype=np.float32)
    b_proj = np.asarray(b_proj, dtype=np.float32)

    nc = _get_nc()
    in_maps = []
    for c in range(N_CORES):
        b = c // 4
        g = c % 4
        qs = slice(g * CL, (g + 1) * CL)
        ks = slice(C + g * CL, C + (g + 1) * CL)
        vs = slice(2 * C + g * CL, 2 * C + (g + 1) * CL)
        in_maps.append({
            "xT": np.ascontiguousarray(x[b].T),
            "w_qk": np.ascontiguousarray(
                np.concatenate([w_qkv[:, qs], w_qkv[:, ks]], axis=1)),
            "b_qk": np.ascontiguousarray(
                np.concatenate([b_qkv[qs], b_qkv[ks]])),
            "w_v": np.ascontiguousarray(w_qkv[:, vs]),
            "b_v": np.ascontiguousarray(b_qkv[vs]),
            "w_pr": np.ascontiguousarray(w_proj[g * CL:(g + 1) * CL, :]),
            "b_pr": b_proj if g == 0 else np.zeros_like(b_proj),
        })

    global _last_in_maps
    _last_in_maps = in_maps
    res = bass_utils.run_bass_kernel_spmd(nc, in_maps, core_ids=list(range(N_CORES)))

    out = np.empty((B, T, C), dtype=np.float32)
    for b in range(B):
        projT = np.concatenate(
            [res.results[4 * b + r]["out_rs"] for r in range(4)], axis=0)
        out[b] = projT.T
    return out


if __name__ == "__main__":
    rng = np.random.RandomState(0)
    ins = {
        "x": rng.randn(B, T, C).astype(np.float32),
        "w_qkv": rng.randn(C, 3 * C).astype(np.float32) / 32,
        "b_qkv": rng.randn(3 * C).astype(np.float32) / 32,
        "w_proj": rng.randn(C, C).astype(np.float32) / 32,
        "b_proj": rng.randn(C).astype(np.float32) / 32,
    }
    y = kernel(**ins)
    print("kernel ran, out shape", y.shape)


# revision 12
# speedup vs baseline: 1.3494x; 1.0642x over previous
"""Causal self-attention (B=2, T=2048, C=1024, H=16) on 8 trn2 NeuronCores.

Sharding: data-parallel over batch (2 groups of 4 cores) x tensor-parallel over
heads (4 heads / core). Per-token-chunk pipeline: for each 512-token chunk the
core computes Q^T/K^T ([ch,tok]) and V ([tok,ch]) slices, causal attention for
its 4 heads (softmax denominator folded into the PV matmul via a ones column),
its slice of the output projection, and a chunked ReduceScatter over the 4-core
group that overlaps the next chunk's compute. Diagonal attention blocks are
computed over a restricted q-window with the triangle masked by a small DVE
multiply, so no full-width mask multiplies and no wasted exp columns.
"""

import sys

for _p in ("/opt/trn_rl_repo",):
    if _p not in sys.path:
        sys.path.append(_p)

import numpy as np
from contextlib import ExitStack

import concourse.bass as bass
import concourse.mybir as mybir
import concourse.tile as tile
from concourse import bass_utils

B, T, C, H = 2, 2048, 1024, 16
D = C // H              # 64
N_CORES = 8
GROUPS = [[0, 1, 2, 3], [4, 5, 6, 7]]
HL = 4                  # heads per core
CL = HL * D             # 256 local channels
KC = C // 128           # 8 contraction chunks of 128
NT = T // 512           # 4 token chunks of 512
F32 = mybir.dt.float32
F32R = mybir.dt.float32r


def _legalize_waits(nc):
    """This walrus build allows at most ONE sync-wait per instruction. Move
    extra waits onto same-engine NoOps inserted just before the instruction."""
    n_split = 0
    for f in nc.m.functions:
        for bb in f.blocks:
            out = []
            for inst in bb.instructions:
                si = inst.sync_info
                waits = list(si.on_wait) if si is not None and si.on_wait else []
                if len(waits) > 1:
                    for i, w in enumerate(waits[:-1]):
                        out.append(
                            mybir.InstNoOp(
                                name=f"wsplit_{inst.name}_{i}",
                                engine=inst.engine,
                                ins=[],
                                outs=[],
                                sync_info=mybir.SyncInfo(on_wait=[w], on_update=[]),
                            )
                        )
                        n_split += 1
                    si.on_wait = [waits[-1]]
                out.append(inst)
            bb.instructions = out
    return n_split


def _build_bass():
    nc = bass.Bass("TRN2", target_bir_lowering=False, debug=False,
                   num_devices=N_CORES)

    xT = nc.dram_tensor("xT", [C, T], F32R, kind="ExternalInput").ap()
    w_qk = nc.dram_tensor("w_qk", [C, 2 * CL], F32R, kind="ExternalInput").ap()
    b_qk = nc.dram_tensor("b_qk", [2 * CL], F32, kind="ExternalInput").ap()
    w_v = nc.dram_tensor("w_v", [C, CL], F32R, kind="ExternalInput").ap()
    b_v = nc.dram_tensor("b_v", [CL], F32R, kind="ExternalInput").ap()
    w_pr = nc.dram_tensor("w_pr", [CL, C], F32R, kind="ExternalInput").ap()
    b_pr = nc.dram_tensor("b_pr", [C], F32, kind="ExternalInput").ap()
    out_rs = nc.dram_tensor("out_rs", [C // 4, T], F32, kind="ExternalOutput").ap()

    with tile.TileContext(nc) as tc:
        with ExitStack() as ctx:
            with nc.allow_low_precision(reason="float32r is 4-byte; full fp32 accumulate"):
                _build_body(ctx, tc, nc, xT, w_qk, b_qk, w_v, b_v, w_pr, b_pr, out_rs)

    _legalize_waits(nc)
    return nc


def _build_body(ctx, tc, nc, xT, w_qk, b_qk, w_v, b_v, w_pr, b_pr, out_rs):
    Exp = mybir.ActivationFunctionType.Exp

    persist = ctx.enter_context(tc.tile_pool(name="persist", bufs=1))
    dram = ctx.enter_context(tc.tile_pool(name="dram", bufs=1, space="DRAM"))

    # ---- constant / bias tiles -------------------------------------------
    b_qk_sb = persist.tile([128, 4], F32, name="b_qk_sb")
    nc.sync.dma_start(b_qk_sb[:], b_qk.rearrange("(m p) -> p m", p=128))
    b_pr_sb = persist.tile([128, 8], F32, name="b_pr_sb")
    nc.sync.dma_start(b_pr_sb[:], b_pr.rearrange("(m p) -> p m", p=128))
    b_v_row = persist.tile([1, CL], F32R, name="b_v_row")
    nc.sync.dma_start(b_v_row[:], b_v.rearrange("(a c) -> a c", a=1))
    ones_sb = persist.tile([128, HL], F32, name="ones_sb")
    nc.gpsimd.memset(ones_sb[:], 1.0)
    ones_f32 = persist.tile([1, 128], F32, name="ones_f32")
    nc.gpsimd.memset(ones_f32[:], 1.0)
    ones_row = persist.tile([1, 128], F32R, name="ones_row")
    nc.vector.tensor_copy(ones_row[:], ones_f32[:])
    # broadcast b_v across partitions via a rank-1 matmul
    b_v_bc = persist.tile([128, CL], F32, name="b_v_bc")
    with tc.tile_pool(name="psInit", bufs=1, space="PSUM") as psI:
        bvp = psI.tile([128, CL], F32)
        nc.tensor.matmul(bvp[:], lhsT=ones_row[:], rhs=b_v_row[:],
                         start=True, stop=True)
        nc.vector.tensor_copy(b_v_bc[:], bvp[:])
    # 0/1 masks for diagonal blocks:
    #   M_tri[x, y] = y - x >= 0            (128x128 lower-left triangle)
    #   M_zt[x, y]  = y - x - 128 >= 0      (zeros strip | triangle, 128x256)
    M_tri = persist.tile([128, 128], F32, name="M_tri")
    nc.gpsimd.memset(M_tri[:], 1.0)
    nc.gpsimd.affine_select(
        out=M_tri[:], in_=M_tri[:], compare_op=mybir.AluOpType.is_ge, fill=0.0,
        base=0, pattern=[[1, 128]], channel_multiplier=-1)
    M_zt = persist.tile([128, 256], F32, name="M_zt")
    nc.gpsimd.memset(M_zt[:], 1.0)
    nc.gpsimd.affine_select(
        out=M_zt[:], in_=M_zt[:], compare_op=mybir.AluOpType.is_ge, fill=0.0,
        base=-128, pattern=[[1, 256]], channel_multiplier=-1)

    # ---- weight + activation loads (xT chunk 0 first so PE starts early) --
    # x chunks rotate through a 16-slot pool (2 chunks resident) with
    # prefetch, instead of pinning all 64KB/partition of xT in SBUF.
    pX = ctx.enter_context(tc.tile_pool(name="pX", bufs=16))
    xT_sb = {}

    def load_x_chunk(n):
        for kc in range(KC):
            xt = pX.tile([128, 512], F32R)
            nc.sync.dma_start(
                xt[:], xT[kc * 128:(kc + 1) * 128, n * 512:(n + 1) * 512])
            xT_sb[kc, n] = xt

    load_x_chunk(0)
    w_qk_sb = []
    for kc in range(KC):
        t = persist.tile([128, 2 * CL], F32R, name=f"w_qk_{kc}")
        nc.sync.dma_start(t[:], w_qk[kc * 128:(kc + 1) * 128, :])
        w_qk_sb.append(t)
    w_v_sb = []
    for kc in range(KC):
        t = persist.tile([128, CL], F32R, name=f"w_v_{kc}")
        nc.sync.dma_start(t[:], w_v[kc * 128:(kc + 1) * 128, :])
        w_v_sb.append(t)
    w_pr_sb = []
    for kc in range(2):
        t = persist.tile([128, C], F32R, name=f"w_pr_{kc}")
        nc.sync.dma_start(t[:], w_pr[kc * 128:(kc + 1) * 128, :])
        w_pr_sb.append(t)
    load_x_chunk(1)

    # ---- persistent intermediates ----------------------------------------
    # QK_sb[m]: m=0,1 -> Q channels (heads 0,1 | 2,3), m=2,3 -> K channels
    QK_sb = [persist.tile([128, T], F32R, name=f"QK_{m}") for m in range(4)]
    # V in [tok, ch] layout, 65 cols/head: col h*65+64 is the ones column
    V_sb = [persist.tile([128, HL * 65], F32R, name=f"V_{t}") for t in range(16)]
    yT_sb = [persist.tile([128, T], F32R, name=f"yT_{i}") for i in range(2)]

    # per-chunk DRAM bounce tensors for the chunked ReduceScatter
    bounce_in = [dram.tile([C, 512], F32, name=f"bounce_in_{n}") for n in range(NT)]
    bounce_rs = [dram.tile([C // 4, 512], F32, name=f"bounce_rs_{n}") for n in range(NT)]

    def q_ap(h):
        return QK_sb[h // 2][(h % 2) * 64:(h % 2) * 64 + 64, :]

    def k_ap(h):
        return QK_sb[2 + h // 2][(h % 2) * 64:(h % 2) * 64 + 64, :]

    # ---- pools ------------------------------------------------------------
    psMM = ctx.enter_context(tc.tile_pool(name="psMM", bufs=2, space="PSUM"))
    psS = ctx.enter_context(tc.tile_pool(name="psS", bufs=2, space="PSUM"))
    psY = ctx.enter_context(tc.tile_pool(name="psY", bufs=2, space="PSUM"))
    pPf = ctx.enter_context(tc.tile_pool(name="pPf", bufs=7))
    pPd = ctx.enter_context(tc.tile_pool(name="pPd", bufs=5))
    pR = ctx.enter_context(tc.tile_pool(name="pR", bufs=2))
    pBC = ctx.enter_context(tc.tile_pool(name="pBC", bufs=2))
    pO = ctx.enter_context(tc.tile_pool(name="pO", bufs=4))

    # ---- emission helpers -------------------------------------------------
    def A_group(n, m):
        """One 128-out-channel group of the Q/K gemm for chunk n."""
        nsl = slice(n * 512, (n + 1) * 512)
        ps = psMM.tile([128, 512], F32)
        for kc in range(KC):
            nc.tensor.matmul(
                ps[:], lhsT=w_qk_sb[kc][:, m * 128:(m + 1) * 128],
                rhs=xT_sb[kc, n][:], start=(kc == 0), stop=(kc == KC - 1))
        nc.vector.tensor_scalar_add(QK_sb[m][:, nsl], ps[:], b_qk_sb[:, m:m + 1])

    def B_group(n, t):
        """One 128-token group of the V gemm for chunk n."""
        ps = psMM.tile([128, 512], F32)
        for kc in range(KC):
            nc.tensor.matmul(
                ps[:, 0:CL], lhsT=xT_sb[kc, n][:, t * 128:(t + 1) * 128],
                rhs=w_v_sb[kc][:], start=(kc == 0), stop=(kc == KC - 1))
        tok = n * 4 + t
        vt = V_sb[tok][:].rearrange("p (h c) -> p h c", h=HL)
        nc.vector.tensor_add(
            vt[:, :, 0:64], ps[:, 0:CL].rearrange("p (h c) -> p h c", h=HL),
            b_v_bc[:].rearrange("p (h c) -> p h c", h=HL))
        nc.vector.tensor_copy(vt[:, :, 64:65].rearrange("p h c -> p (h c)"),
                              ones_sb[:])

    def D_group(n, m):
        """One 128-out-channel group of the output projection for chunk n."""
        nsl = slice(n * 512, (n + 1) * 512)
        ps = psMM.tile([128, 512], F32)
        for kc in range(2):
            nc.tensor.matmul(
                ps[:], lhsT=w_pr_sb[kc][:, m * 128:(m + 1) * 128],
                rhs=yT_sb[kc][:, nsl], start=(kc == 0), stop=(kc == 1))
        o_sb = pO.tile([128, 512], F32)
        if m % 2 == 0:
            nc.scalar.activation(o_sb[:], ps[:],
                                 mybir.ActivationFunctionType.Identity,
                                 bias=b_pr_sb[:, m:m + 1])
        else:
            nc.vector.tensor_scalar_add(o_sb[:], ps[:], b_pr_sb[:, m:m + 1])
        nc.sync.dma_start(bounce_in[n][m * 128:(m + 1) * 128, :], o_sb[:])

    def emit_C(n, fillers):
        """Causal attention for q-chunk n. `fillers` is a list of zero-Act
        matmul-group closures (A/B of chunk n+1) popped into the S-stream so
        the PE stays busy while Act works through the exp backlog."""
        nsl = slice(n * 512, (n + 1) * 512)
        nfull = 4 * n
        pending_tail = [None]

        def emit_tail(h, y_ps):
            rec = pR.tile([1, 512], F32R)
            nc.vector.reciprocal(rec[:], y_ps[64:65, :])
            ps = psMM.tile([128, 512], F32)
            nc.tensor.matmul(ps[0:64, :], lhsT=ones_row[:, 0:64], rhs=rec[:],
                             start=True, stop=True)
            bc = pBC.tile([64, 512], F32R)
            nc.vector.tensor_copy(bc[:], ps[0:64, :])
            nc.vector.tensor_mul(
                yT_sb[h // 2][(h % 2) * 64:(h % 2) * 64 + 64, nsl],
                y_ps[0:64, :], bc[:])

        for h in range(HL):
            y_ps = psY.tile([65, 512], F32)
            quota = (len(fillers) + (HL - h) - 1) // (HL - h) if fillers else 0
            s_allocs = 0

            def after_s_group(pending_tail=pending_tail):
                nonlocal s_allocs, quota
                s_allocs += 1
                if s_allocs == 2 and pending_tail[0] is not None:
                    pending_tail[0]()
                    pending_tail[0] = None
                if s_allocs % 2 == 0 and quota > 0 and fillers:
                    fillers.pop(0)()
                    quota -= 1

            # pass 1: stream S matmuls; Act fills the deep p-tile buffers
            p_full, p_diag = [], []
            for pi in range(nfull // 2):
                j0 = 2 * pi
                s = psS.tile([128, 1024], F32)
                nc.tensor.matmul(
                    s[:, 0:512], lhsT=k_ap(h)[:, j0 * 128:(j0 + 1) * 128],
                    rhs=q_ap(h)[:, nsl], start=True, stop=True)
                nc.tensor.matmul(
                    s[:, 512:1024],
                    lhsT=k_ap(h)[:, (j0 + 1) * 128:(j0 + 2) * 128],
                    rhs=q_ap(h)[:, nsl], start=True, stop=True)
                p = pPf.tile([128, 1024], F32R)
                nc.scalar.activation(p[:], s[:], Exp, scale=0.125)
                p_full.append(p)
                after_s_group()
            for r in range(4):
                j = nfull + r
                w = min(128 * r, 256)
                s = psS.tile([128, 1024], F32)
                nc.tensor.matmul(
                    s[:, w:512], lhsT=k_ap(h)[:, j * 128:(j + 1) * 128],
                    rhs=q_ap(h)[:, n * 512 + w:(n + 1) * 512],
                    start=True, stop=True)
                p = pPd.tile([128, 512], F32R)
                nc.scalar.activation(p[:, w:512], s[:, w:512], Exp, scale=0.125)
                if r < 3:
                    nc.vector.tensor_mul(p[:, 128 * r:128 * (r + 1)],
                                         p[:, 128 * r:128 * (r + 1)], M_tri[:])
                else:
                    nc.vector.tensor_mul(p[:, 256:512], p[:, 256:512], M_zt[:])
                p_diag.append(p)
                after_s_group()
            # pass 2: PV matmuls consume the buffered p tiles in order
            for pi in range(nfull // 2):
                j0 = 2 * pi
                p = p_full[pi]
                nc.tensor.matmul(
                    y_ps[:], lhsT=V_sb[j0][:, h * 65:(h + 1) * 65],
                    rhs=p[:, 0:512], start=(pi == 0), stop=False)
                nc.tensor.matmul(
                    y_ps[:], lhsT=V_sb[j0 + 1][:, h * 65:(h + 1) * 65],
                    rhs=p[:, 512:1024], start=False, stop=False)
            for r in range(4):
                j = nfull + r
                w = min(128 * r, 256)
                p = p_diag[r]
                nc.tensor.matmul(
                    y_ps[:, w:512], lhsT=V_sb[j][:, h * 65:(h + 1) * 65],
                    rhs=p[:, w:512], start=(n == 0 and r == 0), stop=(r == 3))
            pending_tail[0] = (lambda h=h, y_ps=y_ps: emit_tail(h, y_ps))
        pending_tail[0]()

    # ---- software-pipelined main loop ------------------------------------
    for m in range(4):
        A_group(0, m)
    for t in range(4):
        B_group(0, t)
    for n in range(NT):
        if n + 2 < NT:
            load_x_chunk(n + 2)
        fillers = []
        if n + 1 < NT:
            fillers += [lambda m=m: A_group(n + 1, m) for m in range(4)]
            fillers += [lambda t=t: B_group(n + 1, t) for t in range(4)]
        emit_C(n, fillers)
        while fillers:
            fillers.pop(0)()
        for m in range(8):
            D_group(n, m)
        nc.gpsimd.collective_compute(
            "ReduceScatter", mybir.AluOpType.add, replica_groups=GROUPS,
            ins=[bounce_in[n][:]], outs=[bounce_rs[n][:]])
        nc.sync.dma_start(out_rs[:, n * 512:(n + 1) * 512], bounce_rs[n][:])


_NC_CACHE = None


def _get_nc():
    global _NC_CACHE
    if _NC_CACHE is None:
        _NC_CACHE = _build_bass()
    return _NC_CACHE


def kernel(x, w_qkv, b_qkv, w_proj, b_proj, **_kw):
    x = np.asarray(x, dtype=np.float32)
    w_qkv = np.asarray(w_qkv, dtype=np.float32)
    b_qkv = np.asarray(b_qkv, dtype=np.float32)
    w_proj = np.asarray(w_proj, dtype=np.float32)
    b_proj = np.asarray(b_proj, dtype=np.float32)

    nc = _get_nc()
    in_maps = []
    for c in range(N_CORES):
        b = c // 4
        g = c % 4
        qs = slice(g * CL, (g + 1) * CL)
        ks = slice(C + g * CL, C + (g + 1) * CL)
        vs = slice(2 * C + g * CL, 2 * C + (g + 1) * CL)
        in_maps.append({
            "xT": np.ascontiguousarray(x[b].T),
            "w_qk": np.ascontiguousarray(
                np.concatenate([w_qkv[:, qs], w_qkv[:, ks]], axis=1)),
            "b_qk": np.ascontiguousarray(
                np.concatenate([b_qkv[qs], b_qkv[ks]])),
            "w_v": np.ascontiguousarray(w_qkv[:, vs]),
            "b_v": np.ascontiguousarray(b_qkv[vs]),
            "w_pr": np.ascontiguousarray(w_proj[g * CL:(g + 1) * CL, :]),
            "b_pr": b_proj if g == 0 else np.zeros_like(b_proj),
        })

    global _last_in_maps
    _last_in_maps = in_maps
    res = bass_utils.run_bass_kernel_spmd(nc, in_maps, core_ids=list(range(N_CORES)))

    out = np.empty((B, T, C), dtype=np.float32)
    for b in range(B):
        projT = np.concatenate(
            [res.results[4 * b + r]["out_rs"] for r in range(4)], axis=0)
        out[b] = projT.T
    return out


if __name__ == "__main__":
    rng = np.random.RandomState(0)
    ins = {
        "x": rng.randn(B, T, C).astype(np.float32),
        "w_qkv": rng.randn(C, 3 * C).astype(np.float32) / 32,
        "b_qkv": rng.randn(3 * C).astype(np.float32) / 32,
        "w_proj": rng.randn(C, C).astype(np.float32) / 32,
        "b_proj": rng.randn(C).astype(np.float32) / 32,
    }
    y = kernel(**ins)
    print("kernel ran, out shape", y.shape)
